# revision 2
# baseline (speedup 1.0000x reference)
"""Sparse-attention Trainium2 kernel, 8-core SPMD — v3.

Sharding: one head per NeuronCore (8 heads / 8 cores), batch replicated.

v3 structure (vs v2): phases overlap.
- PSUM pools split by role so phases don't contend:
  psA = ps_s (fp16, 1 bank) + ps_t (fp16)   [2 bufs = 2 banks]
  psB = ps_ov (fp32)                        [4 bufs = 4 banks]
  psC = ps_qkv / ps_f / ps_d (fp32)         [2 bufs = 2 banks]
- eb loaded once per i-chunk into a persistent [128, 8192] slab
  (2-buffered) and shared by both batch pairs.
- phase C (out-proj + divide) interleaved per i-chunk so it overlaps
  the next chunk's attention.
- phase-A PSUM evacuations split DVE/ACT; exp(bias) multiply split
  DVE/GPSIMD; phase-C scale split ACT/DVE.
- RoPE via pre-swapped extra weight columns (no strided pair-swap).
"""

import numpy as np

B, N, C = 4, 2048, 512
HEADS, D = 8, 64
NCORES = 8
ROPE_THETA = 10000.0

NT = N // 128       # 16 seq tiles of 128
IC = N // 512       # 4 i-chunks of 512
F16 = np.float16

_cache = {}


def _rope_tables():
    inv = 1.0 / (ROPE_THETA ** (np.arange(0, D, 2, dtype=np.float64) / D))
    freqs = np.arange(N, dtype=np.float64)[:, None] * inv            # [N, 32]
    freqs = np.repeat(freqs, 2, axis=-1)                             # [N, 64]
    cos = np.cos(freqs)
    sin = np.sin(freqs)
    sin_signed = sin.copy()
    sin_signed[:, 0::2] *= -1.0
    scale = D ** -0.5
    return cos, sin_signed, scale


def _sb_tab2(t):
    # [N, D] -> SBUF [128, NT*2*D]: tile nt at cols nt*128, D cols
    # duplicated for the two batch halves.
    t2 = np.concatenate([t, t], axis=1)                              # [N, 2D]
    return np.ascontiguousarray(
        t2.reshape(NT, 128, 2 * D).transpose(1, 0, 2)
        .reshape(128, NT * 2 * D).astype(F16))


def _build():
    import concourse.bacc as bacc
    import concourse.mybir as mybir
    import concourse.tile as tile

    exp_fn = mybir.ActivationFunctionType.Exp
    fp16 = mybir.dt.float16
    fp32 = mybir.dt.float32

    nc = bacc.Bacc(None)

    xT = nc.declare_dram_parameter("xT", [B, C, N], fp16, isOutput=False)
    # per c-chunk slab: q64 | k64 | v64 | qsw64 | ksw64  (320 cols)
    wqkvT = nc.declare_dram_parameter("wqkvT", [C, 320], fp16, isOutput=False)
    woT2 = nc.declare_dram_parameter("woT2", [128, C], fp16, isOutput=False)
    eb = nc.declare_dram_parameter("eb", [N, N], fp16, isOutput=False)
    out_ext = nc.declare_dram_parameter("out", [B, N, C], fp16, isOutput=True)

    cos, sin_signed, scale = _rope_tables()
    cosq_h = nc.inline_tensor(_sb_tab2(cos * scale), name="cosq")
    sinq_h = nc.inline_tensor(_sb_tab2(sin_signed * scale), name="sinq")
    cosk_h = nc.inline_tensor(_sb_tab2(cos), name="cosk")
    sink_h = nc.inline_tensor(_sb_tab2(sin_signed), name="sink")
    ident_h = nc.inline_tensor(np.eye(128, dtype=F16), name="ident")

    with tile.TileContext(nc) as tc:
        with (
            tc.tile_pool(name="const", bufs=1) as cpool,
            tc.tile_pool(name="xt", bufs=16) as xtp,
            tc.tile_pool(name="qk", bufs=1) as qkp,
            tc.tile_pool(name="rope", bufs=3) as rpp,
            tc.tile_pool(name="ptp", bufs=4) as ptp,
            tc.tile_pool(name="ebp", bufs=2) as ebp,
            tc.tile_pool(name="outsb", bufs=4) as osb,
            tc.tile_pool(name="psA", bufs=2, space="PSUM") as psA,
            tc.tile_pool(name="psB", bufs=2, space="PSUM") as psB,
            tc.tile_pool(name="psC", bufs=2, space="PSUM") as psC,
        ):
            # ---- persistent SBUF tensors ----
            cosq = cpool.tile([128, NT * 128], fp16, tag="cosq")
            sinq = cpool.tile([128, NT * 128], fp16, tag="sinq")
            cosk = cpool.tile([128, NT * 128], fp16, tag="cosk")
            sink = cpool.tile([128, NT * 128], fp16, tag="sink")
            ident = cpool.tile([128, 128], fp16, tag="ident")
            wq = cpool.tile([128, 4 * 320], fp16, tag="wq")   # 4 c-chunks
            wo = cpool.tile([128, C], fp16, tag="wo")
            nc.sync.dma_start(cosq[:], cosq_h[:])
            nc.sync.dma_start(sinq[:], sinq_h[:])
            nc.sync.dma_start(cosk[:], cosk_h[:])
            nc.sync.dma_start(sink[:], sink_h[:])
            nc.sync.dma_start(ident[:], ident_h[:])
            for cc in range(4):
                nc.sync.dma_start(
                    wq[:, cc * 320:(cc + 1) * 320],
                    wqkvT[cc * 128:(cc + 1) * 128, :])
            nc.sync.dma_start(wo[:], woT2[:])

            qT = [qkp.tile([128, N], fp16, tag=f"qT{p}", name=f"qT{p}")
                  for p in range(2)]
            kT = [qkp.tile([128, N], fp16, tag=f"kT{p}", name=f"kT{p}")
                  for p in range(2)]
            vsb = [qkp.tile([128, NT * (D + 1)], fp16, tag=f"v{b}",
                            name=f"v{b}") for b in range(B)]
            for b in range(B):
                nc.gpsimd.memset(vsb[b][:], 1.0)
            outT = [qkp.tile([64, N], fp16, tag=f"outT{b}", name=f"outT{b}")
                    for b in range(B)]
            # denominator staging: batch b's row lives at partition 32*b
            dstage = qkp.tile([128, N], fp16, tag="dstage")
            nc.gpsimd.memset(dstage[:], 1.0)
            rs_r = qkp.tile([128, 64], fp32, tag="rs")    # recip rowsums
            rs_raw = qkp.tile([128, 64], fp32, tag="rsraw")

            # ---- phase A: qkv proj + rope + paired transposes ----
            for pr in range(2):
                bpair = (2 * pr, 2 * pr + 1)
                xt = [xtp.tile([128, N], fp16, tag="xt",
                               name=f"xt{pr}_{i}") for i in range(8)]
                for i, b in enumerate(bpair):
                    for cc in range(4):
                        nc.sync.dma_start(
                            xt[4 * i + cc][:],
                            xT[b, cc * 128:(cc + 1) * 128, :])
                for nt in range(NT):
                    nsl = slice(nt * 128, (nt + 1) * 128)
                    csl = slice(nt * 128, (nt + 1) * 128)
                    qkv = rpp.tile([128, 640], fp16, tag="qkv")
                    for i in range(2):
                        ps_qkv = psC.tile([128, 320], fp32, tag="psC",
                                          name="ps_qkv")
                        for cc in range(4):
                            nc.tensor.matmul(
                                ps_qkv[:],
                                xt[4 * i + cc][:, nsl],
                                wq[:, cc * 320:(cc + 1) * 320],
                                start=(cc == 0), stop=(cc == 3))
                        if i == 0:
                            nc.scalar.copy(qkv[:, 0:320], ps_qkv[:])
                        else:
                            nc.vector.tensor_copy(qkv[:, 320:640], ps_qkv[:])
                    q3 = qkv[:].rearrange("p (h x) -> p h x", h=2)
                    for i, b in enumerate(bpair):
                        vdsl = slice(nt * (D + 1), nt * (D + 1) + D)
                        nc.vector.tensor_copy(
                            vsb[b][:, vdsl],
                            qkv[:, 320 * i + 128:320 * i + 192])
                    qpair = rpp.tile([128, 128], fp16, tag="qpair")
                    kpair = rpp.tile([128, 128], fp16, tag="kpair")
                    tmp = rpp.tile([128, 128], fp16, tag="tmp")
                    for (pair, o_d, o_s, ct, st) in (
                            (qpair, 0, 192, cosq, sinq),
                            (kpair, 64, 256, cosk, sink)):
                        p3 = pair[:].rearrange("p (h x) -> p h x", h=2)
                        t3 = tmp[:].rearrange("p (h x) -> p h x", h=2)
                        c3 = ct[:, csl].rearrange("p (h x) -> p h x", h=2)
                        s3 = st[:, csl].rearrange("p (h x) -> p h x", h=2)
                        nc.vector.tensor_mul(p3, q3[:, :, o_d:o_d + 64], c3)
                        nc.vector.tensor_mul(t3, q3[:, :, o_s:o_s + 64], s3)
                        nc.vector.tensor_add(pair[:], pair[:], tmp[:])
                    for (j, (pair, dst)) in enumerate(
                            ((qpair, qT[pr]), (kpair, kT[pr]))):
                        ps_t = psA.tile([128, 1024], fp16, tag="psA",
                                        name="ps_t")
                        nc.tensor.transpose(ps_t[:, 0:128], pair[:], ident[:])
                        if j == 0:
                            nc.scalar.copy(dst[:, nsl], ps_t[:, 0:128])
                        else:
                            nc.vector.tensor_copy(dst[:, nsl], ps_t[:, 0:128])

            # ---- phase B + C, interleaved per i-chunk ----
            mulc = 0
            for ic in range(IC):
                isl = slice(ic * 512, (ic + 1) * 512)
                # eb slab for this i-chunk: [j=2048, i=512] as [128, 16*512]
                ebt = ebp.tile([128, NT * 512], fp16, tag="eb")
                for jt in range(NT):
                    nc.sync.dma_start(
                        ebt[:, jt * 512:(jt + 1) * 512],
                        eb[jt * 128:(jt + 1) * 128, isl])
                for pr in range(2):
                    ps_ov = [psB.tile([128, 512], fp32, tag="psB",
                                      name=f"ps_ov{2 * pr + bh}")
                             for bh in range(2)]
                    for jp in range(NT // 2):
                        for bh in range(2):
                            b = 2 * pr + bh
                            po = 64 * bh
                            ps_s = psA.tile([128, 1024], fp32, tag="psA",
                                            name="ps_s")
                            for hh in range(2):
                                jt = 2 * jp + hh
                                jsl = slice(jt * 128, (jt + 1) * 128)
                                nc.tensor.matmul(
                                    ps_s[:, hh * 512:(hh + 1) * 512],
                                    kT[pr][po:po + 64, jsl],
                                    qT[pr][po:po + 64, isl],
                                    start=True, stop=True)
                            pt = ptp.tile([128, 1024], fp16, tag="pt")
                            nc.scalar.activation(pt[:], ps_s[:], func=exp_fn)
                            esl = slice(2 * jp * 512, (2 * jp + 2) * 512)
                            if mulc % 3 == 0:
                                nc.gpsimd.tensor_mul(pt[:], pt[:], ebt[:, esl])
                            else:
                                nc.vector.tensor_mul(pt[:], pt[:], ebt[:, esl])
                            mulc += 1
                            for hh in range(2):
                                jt = 2 * jp + hh
                                nc.tensor.matmul(
                                    ps_ov[bh][0:65, :],
                                    vsb[b][:, jt * 65:jt * 65 + 65],
                                    pt[:, hh * 512:(hh + 1) * 512],
                                    start=(jp == 0 and hh == 0),
                                    stop=(jp == NT // 2 - 1 and hh == 1),
                                    skip_group_check=True)
                    for bh in range(2):
                        b = 2 * pr + bh
                        nc.vector.tensor_copy(outT[b][0:64, isl],
                                              ps_ov[bh][0:64, :])
                        nc.vector.tensor_copy(
                            dstage[32 * b:32 * b + 1, isl],
                            ps_ov[bh][64:65, :])
                # denominator rows -> columns; rs col = it*4+b
                for t in range(4):
                    it = ic * 4 + t
                    ps_d = psC.tile([128, 512], fp16, tag="psC", name="ps_d")
                    nc.tensor.transpose(
                        ps_d[:, 0:128],
                        dstage[:, it * 128:(it + 1) * 128],
                        ident[:])
                    nc.vector.tensor_copy(
                        rs_raw[:, it * 4:(it + 1) * 4],
                        ps_d[:, 0:128].rearrange(
                            "p (b s) -> p b s", s=32)[:, :, 0])
                nc.vector.reciprocal(rs_r[:, ic * 16:(ic + 1) * 16],
                                     rs_raw[:, ic * 16:(ic + 1) * 16])
                # phase C for this i-chunk
                for b in range(B):
                    for t in range(4):
                        it = ic * 4 + t
                        ps_f = psC.tile([128, 512], fp32, tag="psC",
                                        name="ps_f")
                        nc.tensor.matmul(
                            ps_f[:],
                            outT[b][0:64, it * 128:(it + 1) * 128],
                            wo[0:64, :], start=True, stop=True)
                        osb_t = osb.tile([128, C], fp16, tag="osb")
                        c1 = it * 4 + b
                        if (b + t) % 2 == 0:
                            nc.scalar.mul(osb_t[:], ps_f[:],
                                          rs_r[:, c1:c1 + 1])
                        else:
                            nc.vector.tensor_scalar_mul(
                                osb_t[:], ps_f[:], rs_r[:, c1:c1 + 1])
                        nc.sync.dma_start(
                            out_ext[b, it * 128:(it + 1) * 128, :], osb_t[:])
    nc.finalize()
    return nc


def kernel(x, pos_bias, w_qkv, w_out):
    from concourse.bass_utils import run_bass_kernel_spmd

    if "nc" not in _cache:
        _cache["nc"] = _build()
    nc = _cache["nc"]

    xT = np.ascontiguousarray(x.transpose(0, 2, 1)).astype(F16)
    in_maps = []
    for h in range(NCORES):
        hs = slice(h * D, (h + 1) * D)
        wqh = w_qkv[hs]                                   # [64, C]
        wkh = w_qkv[C + h * D:C + (h + 1) * D]
        wvh = w_qkv[2 * C + h * D:2 * C + (h + 1) * D]
        wqsw = wqh.reshape(D // 2, 2, C)[:, ::-1, :].reshape(D, C)
        wksw = wkh.reshape(D // 2, 2, C)[:, ::-1, :].reshape(D, C)
        wslab = np.concatenate([wqh, wkh, wvh, wqsw, wksw], axis=0)  # [320,C]
        wqkvT = np.ascontiguousarray(wslab.T).astype(F16)            # [C,320]
        woT = np.ascontiguousarray(w_out[:, hs].T).astype(F16)       # [64, C]
        woT2 = np.concatenate([woT, woT], axis=0)                    # [128, C]
        ebm = np.exp(pos_bias[h].T.astype(np.float64)).astype(F16)
        in_maps.append({"xT": xT, "wqkvT": wqkvT, "woT2": woT2, "eb": ebm})

    _cache["in_maps"] = in_maps
    res = run_bass_kernel_spmd(nc, in_maps, core_ids=list(range(NCORES)))
    _cache["res"] = res
    out = np.zeros((B, N, C), np.float32)
    for i in range(NCORES):
        out += res.results[i]["out"].astype(np.float32)
    return out
